# revision 3
# baseline (speedup 1.0000x reference)
"""BlockCrossAttention TRN2 Bass kernel — 8-core SPMD, no collectives.

Sharding: core c => batch b = c//4, block-quarter q = c%4.  Each core
pools its 2048 decoder tokens into 128 blocks, computes K/V for its
batch over a mask-compacted encoder sequence, runs attention for all
16 q-heads over its 128 blocks, output-projects, and writes block-level
output rows [128, 1024].  Host broadcasts block rows back to token
level and concatenates.

Key optimizations vs v1:
  * All inputs uploaded as bf16 (halves DRAM->SBUF traffic, removes all
    on-device f32->bf16 casts).
  * Encoder tokens compacted by the attention mask on the host (a pure
    gather; ~2056 of 4096 survive, padded to LKEEP=2304).  Masked
    tokens contribute exp(-1e9)==0 in the reference, so dropping them
    is exact; a per-token validity column in V provides the softmax
    denominator (padding rows contribute K=0 -> exp(0)=1 but valid=0).
  * Scores matmuls are 64-contraction row-tiled pairs (kv-head g even
    on PE rows 0:63, g odd on 64:127) which the PE runs concurrently.
  * exp() is issued as [128, 1024] ACT sweeps straight out of PSUM
    (two kv-groups per sweep) to amortize the ~352-cycle ACT overhead.
  * Attention is split into two kv-group passes so the PSUM budget
    (8 banks) fits: pass A (groups 0,1) pipelines with the K^T/V
    projection matmuls; pass B (groups 2,3) runs after.

Numerics: projections and attention weights bf16, accumulation f32,
softmax exp in f32 on ACT.  Pooling is a SUM over 16 tokens; the /16
is folded into the exp scale (1/(16*sqrt(64))).
"""
import sys

sys.path.insert(0, "/opt/trn_rl_repo")

import numpy as np
import ml_dtypes

import concourse.bass as bass
import concourse.tile as tile
from concourse import bacc, mybir
from concourse.bass import ts
from concourse.bass_utils import run_bass_kernel_spmd
from concourse.masks import make_identity

F32 = mybir.dt.float32
BF16 = mybir.dt.bfloat16

BF16NP = ml_dtypes.bfloat16

# problem constants (hardcoded per contract)
B, LDEC, LENC, D = 2, 8192, 4096, 1024
BLOCK, H, KV, DH = 16, 16, 4, 64
NB = LDEC // BLOCK            # 512 blocks per batch
NCORES = 8
TOK = LDEC // 4               # 2048 decoder tokens per core
NBQ = NB // 4                 # 128 blocks per core
KD = 8                        # 128-wide chunks of D
LKEEP = 2304                  # compacted+padded encoder length (18*128)
NCH = LKEEP // 128            # 18 chunks of 128 enc tokens
# pooled is a SUM over 16 tokens (not mean); fold /16 into the exp scale
SCALE = float(1.0 / (np.sqrt(np.float32(DH)).astype(np.float32) * BLOCK))

_CACHE = {}


def _build():
    nc = bacc.Bacc("TRN2", target_bir_lowering=False, debug=False,
                   num_devices=NCORES)
    hs = nc.dram_tensor("hs", [TOK, D], BF16, kind="ExternalInput").ap()
    encT = nc.dram_tensor("encT", [D, LKEEP], BF16, kind="ExternalInput").ap()
    validpm = nc.dram_tensor("validpm", [128, NCH], F32,
                             kind="ExternalInput").ap()
    wq = nc.dram_tensor("wq", [D, H * DH], BF16, kind="ExternalInput").ap()
    wk = nc.dram_tensor("wk", [D, KV * DH], BF16, kind="ExternalInput").ap()
    wv = nc.dram_tensor("wv", [D, KV * DH], BF16, kind="ExternalInput").ap()
    wo = nc.dram_tensor("wo", [H * DH, D], BF16, kind="ExternalInput").ap()
    outb = nc.dram_tensor("outb", [NBQ, D], F32, kind="ExternalOutput").ap()

    with tile.TileContext(nc) as tc:
        _body(nc, tc, hs, encT, validpm, wq, wk, wv, wo, outb)
    nc.compile()
    return nc


def _body(nc, tc, hs, encT, validpm, wq, wk, wv, wo, outb):
    from contextlib import ExitStack
    with ExitStack() as ctx:
        pool = lambda name, bufs, **kw: ctx.enter_context(
            tc.tile_pool(name=name, bufs=bufs, **kw))

        # ---- long-lived SBUF pools ----
        constp = pool("const", 1)
        wkp = pool("wkp", KD)
        encp = pool("encp", KD)
        wqp = pool("wqp", KD)
        wvp = pool("wvp", KD)
        wop = pool("wop", KD)
        qpp = pool("qpp", 2)
        ktp = pool("ktp", 2)
        v5p = pool("v5p", NCH)
        otp = pool("otp", KD)
        smallp = pool("small", 8)

        # ---- constants ----
        ident = constp.tile([128, 128], BF16)
        make_identity(nc, ident[:])
        vstage = constp.tile([128, NCH], F32)
        nc.sync.dma_start(vstage[:], validpm[:])
        validbf = constp.tile([128, NCH], BF16)
        nc.vector.tensor_copy(validbf[:], vstage[:])

        # ---- input DMAs (emission order ~ priority) ----
        # wk first (K^T projection is the first big PE phase), then enc
        # chunk-major so KT chunk 0 unblocks early, then wq/hs (Q path),
        # then wv, wo last (needed only at the end).
        wk_sb = []
        for k in range(KD):
            t = wkp.tile([128, KV * DH], BF16, tag="wk", name=f"wk{k}")
            nc.sync.dma_start(t[:], wk[ts(k, 128), :])
            wk_sb.append(t)
        enc_sb = [encp.tile([128, LKEEP], BF16, tag="enc", name=f"enc{k}")
                  for k in range(KD)]
        # chunk-major DMA: all 8 D-chunks of enc columns [0:512) land first
        for ce in range(5):
            c0, c1 = 512 * ce, min(512 * (ce + 1), LKEEP)
            for k in range(KD):
                nc.sync.dma_start(enc_sb[k][:, c0:c1],
                                  encT[ts(k, 128), c0:c1])
        wq_sb = []
        for k in range(KD):
            t = wqp.tile([128, H * DH], BF16, tag="wq", name=f"wq{k}")
            nc.sync.dma_start(t[:], wq[ts(k, 128), :])
            wq_sb.append(t)
        wv_sb = []
        for k in range(KD):
            t = wvp.tile([128, KV * DH], BF16, tag="wv", name=f"wv{k}")
            nc.sync.dma_start(t[:], wv[ts(k, 128), :])
            wv_sb.append(t)
        wo_sb = []
        for t_ in range(KD):
            t = wop.tile([128, D], BF16, tag="wo", name=f"wo{t_}")
            nc.sync.dma_start(t[:], wo[ts(t_, 128), :])
            wo_sb.append(t)

        # ---- pooling: pooled[p, d] = sum_j hs[16p + j, d]  (bf16) ----
        pooled = constp.tile([128, D], BF16)
        hsr = hs.rearrange("(p j) d -> p j d", j=BLOCK)
        with tc.tile_pool(name="jbig", bufs=2) as jbig, \
             tc.tile_pool(name="padd", bufs=1) as padd:
            j0 = jbig.tile([128, 8 * D], BF16, tag="jb", name="j0")
            nc.sync.dma_start(j0[:].rearrange("p (j d) -> p j d", d=D),
                              hsr[:, 0:8, :])
            j1 = jbig.tile([128, 8 * D], BF16, tag="jb", name="j1")
            nc.sync.dma_start(j1[:].rearrange("p (j d) -> p j d", d=D),
                              hsr[:, 8:16, :])
            s1 = padd.tile([128, 8 * D], BF16, tag="s1")
            # split the big first-level add across DVE and GPSIMD
            nc.vector.tensor_add(s1[:, 0:4 * D], j0[:, 0:4 * D],
                                 j1[:, 0:4 * D])
            nc.gpsimd.tensor_add(s1[:, 4 * D:8 * D], j0[:, 4 * D:8 * D],
                                 j1[:, 4 * D:8 * D])
            s2 = padd.tile([128, 4 * D], BF16, tag="s2")
            nc.vector.tensor_add(s2[:], s1[:, 0:4 * D], s1[:, 4 * D:8 * D])
            s3 = padd.tile([128, 2 * D], BF16, tag="s3")
            nc.vector.tensor_add(s3[:], s2[:, 0:2 * D], s2[:, 2 * D:4 * D])
            nc.vector.tensor_add(pooled[:], s3[:, 0:D], s3[:, D:2 * D])

        # ---- Q path: transpose pooled, project, pack by kv-group ----
        # qpair[mm] is [128, 512] bf16: partitions 0:64 = dh of kv-group
        # 2mm, 64:128 = kv-group 2mm+1; free = 4 q-heads x 128 blocks
        # (head h = 4g + j at free j*128:(j+1)*128 of its group's half).
        qpair = [qpp.tile([128, 4 * NBQ], BF16, tag=f"qp{mm}",
                          name=f"qpair{mm}") for mm in range(2)]
        with tc.tile_pool(name="tpt", bufs=KD) as tptp, \
             tc.tile_pool(name="ptr", bufs=2, space="PSUM") as ptr, \
             tc.tile_pool(name="pq", bufs=2, space="PSUM") as pq:
            tpT = []
            for k in range(KD):
                # bf16 PSUM tile padded to a full 2KB bank
                ps = ptr.tile([128, 1024], BF16, tag="ptr", name=f"ptr{k}")
                nc.tensor.transpose(ps[:, 0:128], pooled[:, ts(k, 128)],
                                    ident[:])
                tb = tptp.tile([128, 128], BF16, tag="tpT", name=f"tpT{k}")
                nc.vector.tensor_copy(tb[:], ps[:, 0:128])
                tpT.append(tb)
            for m in range(8):
                ps = pq.tile([128, 512], F32, tag="pq", name=f"pq{m}")
                for k in range(KD):
                    nc.tensor.matmul(ps[:, 0:128],
                                     wq_sb[k][:, ts(m, 128)], tpT[k][:],
                                     start=(k == 0), stop=(k == KD - 1))
                for half in range(2):
                    h = 2 * m + half
                    g, j = h // 4, h % 4
                    nc.vector.tensor_copy(
                        qpair[h // 8][ts(g % 2, 64), ts(j, 128)],
                        ps[ts(half, 64), 0:128])

        # ---- KT sbuf tiles [128, LKEEP]: partitions = dh of kv-pair ----
        KTs = [ktp.tile([128, LKEEP], BF16, tag=f"kt{mm}", name=f"KTs{mm}")
               for mm in range(2)]
        V5s = [v5p.tile([128, KV * (DH + 1)], BF16, tag="v5", name=f"v5_{c}")
               for c in range(NCH)]
        OTp = [otp.tile([128, NBQ], BF16, tag="ot", name=f"ot{t}")
               for t in range(KD)]

        # attention accumulators for pass A (kv groups 0,1) — outlive the
        # KT/V psum pools below
        with tc.tile_pool(name="pavA", bufs=2, space="PSUM") as pavA:
            avA = [pavA.tile([128, 512], F32, tag="avA", name=f"avA{g}")
                   for g in range(2)]
            eXap = pool("eXa", 2)

            # ===== pass A: KT/V projections pipelined with attention on
            # kv groups 0,1 =====
            with tc.tile_pool(name="pkt", bufs=2, space="PSUM") as pkt, \
                 tc.tile_pool(name="pv", bufs=2, space="PSUM") as pv, \
                 tc.tile_pool(name="psca", bufs=1, space="PSUM") as psca:
                for c in range(NCH):
                    # K^T for a 512-wide enc chunk every 4th iteration
                    if c % 4 == 0 and c // 4 < 5:
                        ce = c // 4
                        c0, c1 = 512 * ce, min(512 * (ce + 1), LKEEP)
                        w = c1 - c0
                        for mk in range(2):
                            ps = pkt.tile([128, 512], F32, tag="pkt",
                                          name=f"pkt{ce}_{mk}")
                            for k in range(KD):
                                nc.tensor.matmul(
                                    ps[:, 0:w], wk_sb[k][:, ts(mk, 128)],
                                    enc_sb[k][:, c0:c1],
                                    start=(k == 0), stop=(k == KD - 1))
                            nc.vector.tensor_copy(KTs[mk][:, c0:c1],
                                                  ps[:, 0:w])
                    # V for this 128-token chunk (all 4 kv heads + valid)
                    ps = pv.tile([128, 512], F32, tag="pv", name=f"pv{c}")
                    for k in range(KD):
                        nc.tensor.matmul(ps[:, 0:KV * DH],
                                         enc_sb[k][:, ts(c, 128)],
                                         wv_sb[k][:],
                                         start=(k == 0), stop=(k == KD - 1))
                    t5 = V5s[c]
                    t5r = t5[:].rearrange("p (g x) -> p g x", x=DH + 1)
                    psr = ps[:, 0:KV * DH].rearrange("p (g x) -> p g x",
                                                     x=DH)
                    nc.vector.tensor_copy(t5r[:, :, 0:DH], psr)
                    nc.vector.tensor_copy(
                        t5r[:, :, DH:DH + 1],
                        validbf[:, c:c + 1].broadcast_to((128, KV, 1)))

                    # attention pass A on kv groups 0,1 (mm=0)
                    sc = psca.tile([128, 1024], F32, tag="sca",
                                   name=f"scA{c}")
                    nc.tensor.matmul(sc[:, 0:512],
                                     KTs[0][0:64, ts(c, 128)],
                                     qpair[0][0:64, :],
                                     start=True, stop=True)
                    nc.tensor.matmul(sc[:, 512:1024],
                                     KTs[0][64:128, ts(c, 128)],
                                     qpair[0][64:128, :],
                                     start=True, stop=True)
                    eX = eXap.tile([128, 1024], BF16, tag="eXa",
                                   name=f"eXa{c}")
                    nc.scalar.activation(eX[:], sc[:],
                                         mybir.ActivationFunctionType.Exp,
                                         bias=0.0, scale=SCALE)
                    for g in range(2):
                        nc.tensor.matmul(
                            avA[g][0:DH + 1, :],
                            V5s[c][:, ts(g, DH + 1)],
                            eX[:, ts(g, 512)],
                            start=(c == 0), stop=(c == NCH - 1))

            # ===== pass B: attention on kv groups 2,3 =====
            with tc.tile_pool(name="pavB", bufs=2, space="PSUM") as pavB:
                avB = [pavB.tile([128, 512], F32, tag="avB",
                                 name=f"avB{g}") for g in range(2)]
                with tc.tile_pool(name="pscb", bufs=2, space="PSUM") as pscb:
                    for c in range(NCH):
                        sc = pscb.tile([128, 1024], F32, tag="scb",
                                       name=f"scB{c}")
                        nc.tensor.matmul(sc[:, 0:512],
                                         KTs[1][0:64, ts(c, 128)],
                                         qpair[1][0:64, :],
                                         start=True, stop=True)
                        nc.tensor.matmul(sc[:, 512:1024],
                                         KTs[1][64:128, ts(c, 128)],
                                         qpair[1][64:128, :],
                                         start=True, stop=True)
                        eX = eXap.tile([128, 1024], BF16, tag="eXa",
                                       name=f"eXb{c}")
                        nc.scalar.activation(
                            eX[:], sc[:],
                            mybir.ActivationFunctionType.Exp,
                            bias=0.0, scale=SCALE)
                        for gg in range(2):
                            nc.tensor.matmul(
                                avB[gg][0:DH + 1, :],
                                V5s[c][:, ts(2 + gg, DH + 1)],
                                eX[:, ts(gg, 512)],
                                start=(c == 0), stop=(c == NCH - 1))

                # ---- normalize: OTp[t] = [head 2t | head 2t+1] x blocks ----
                for g in range(4):
                    av = avA[g] if g < 2 else avB[g - 2]
                    rec = smallp.tile([1, 512], F32, tag="rec",
                                      name=f"rec{g}")
                    nc.vector.reciprocal(rec[:], av[DH:DH + 1, :])
                    recb = smallp.tile([DH, 512], F32, tag="recb",
                                       name=f"recb{g}")
                    nc.gpsimd.partition_broadcast(recb[:], rec[:])
                    for j in range(4):
                        t, half = 2 * g + j // 2, j % 2
                        nc.vector.tensor_mul(
                            OTp[t][ts(half, 64), :],
                            av[0:DH, ts(j, 128)],
                            recb[:, ts(j, 128)])

                # ---- output projection ----
                with tc.tile_pool(name="outsb", bufs=1) as outsbp, \
                     tc.tile_pool(name="po", bufs=2, space="PSUM") as po:
                    osb = outsbp.tile([128, D], F32)
                    for n in range(2):
                        ps = po.tile([128, 512], F32, tag="po",
                                     name=f"po{n}")
                        for t in range(KD):
                            nc.tensor.matmul(ps[:], OTp[t][:],
                                             wo_sb[t][:, ts(n, 512)],
                                             start=(t == 0),
                                             stop=(t == KD - 1))
                        nc.vector.tensor_copy(osb[:, ts(n, 512)], ps[:])
                    nc.sync.dma_start(outb[:], osb[:])


def prepare_in_maps(hidden_states, encoder_hidden_states, attention_mask,
                    Wq, Wk, Wv, Wo):
    """Host-side prep: bf16 casts, enc transpose + mask compaction."""
    hs = np.asarray(hidden_states, dtype=np.float32)
    enc = np.asarray(encoder_hidden_states, dtype=np.float32)
    mask = np.asarray(attention_mask)
    wq_bf = np.ascontiguousarray(np.asarray(Wq, np.float32).astype(BF16NP))
    wk_bf = np.ascontiguousarray(np.asarray(Wk, np.float32).astype(BF16NP))
    wv_bf = np.ascontiguousarray(np.asarray(Wv, np.float32).astype(BF16NP))
    wo_bf = np.ascontiguousarray(np.asarray(Wo, np.float32).astype(BF16NP))

    encT_bf, validpm = [], []
    for b in range(B):
        idx = np.nonzero(mask[b] != 0)[0]
        n = idx.size
        assert n <= LKEEP, f"kept {n} > LKEEP {LKEEP}"
        encC = np.zeros((LKEEP, D), dtype=BF16NP)
        encC[:n] = enc[b][idx].astype(BF16NP)
        encT_bf.append(np.ascontiguousarray(encC.T))
        v = np.zeros(LKEEP, dtype=np.float32)
        v[:n] = 1.0
        validpm.append(np.ascontiguousarray(v.reshape(NCH, 128).T))

    in_maps = []
    for c in range(NCORES):
        b, q = c // 4, c % 4
        in_maps.append({
            "hs": np.ascontiguousarray(
                hs[b, q * TOK:(q + 1) * TOK].astype(BF16NP)),
            "encT": encT_bf[b],
            "validpm": validpm[b],
            "wq": wq_bf,
            "wk": wk_bf,
            "wv": wv_bf,
            "wo": wo_bf,
        })
    return in_maps


def kernel(hidden_states, encoder_hidden_states, attention_mask, Wq, Wk, Wv, Wo):
    if "nc" not in _CACHE:
        _CACHE["nc"] = _build()
    nc = _CACHE["nc"]

    in_maps = prepare_in_maps(hidden_states, encoder_hidden_states,
                              attention_mask, Wq, Wk, Wv, Wo)
    res = run_bass_kernel_spmd(nc, in_maps, list(range(NCORES)),
                               **_CACHE.get("run_kwargs", {}))
    _CACHE["last_result"] = res
    blocks = np.empty((B, NB, D), dtype=np.float32)
    for c in range(NCORES):
        b, q = c // 4, c % 4
        blocks[b, q * NBQ:(q + 1) * NBQ] = res.results[c]["outb"]
    out = np.repeat(blocks, BLOCK, axis=1)
    return out
